# revision 4
# baseline (speedup 1.0000x reference)
"""Trainium2 Bass kernel for GNN message passing (gather + segment_sum).

out[i] = sum_{e: dst[e]==i} x[src[e]]   with x [100000, 64] f32,
edge_index [2, 1600000] int64.

Strategy (8 NeuronCores, SPMD):
  - Destination nodes sharded across cores: core c owns dst rows
    [c*12500, (c+1)*12500), padded to a 12544-row output slab.
  - x is staged in HBM per core as x_dev [100004, 64]: 4 source chunks of
    25000 rows, each followed by one zero row (pad target). Chunking keeps
    gather indices within int16 range required by dma_gather.
  - Host pre-sorts edges by (dst-core, src-chunk, dst) and builds padded
    "slot" lists: pass 1 gives every node 4 slots per chunk; nodes with
    more than 4 in-edges from a chunk overflow into compacted extra passes
    (4 more slots each), whose partial sums are dma_scatter_add-ed (CCE
    read-modify-write) into the output slab.
  - Device: dma_gather streams 256B rows from HBM into SBUF staging tiles;
    the vector engine reduces the 4-slot planes with strided-AP adds; the
    result is written back with one contiguous DMA per node tile.
"""

import sys

if "/opt/trn_rl_repo" not in sys.path:
    sys.path.insert(0, "/opt/trn_rl_repo")

import numpy as np

N = 100000
D = 64
N_CORES = 8
ROWS_PER_CORE = N // N_CORES            # 12500
NODE_TILE = 1792                        # 14 groups of 128 nodes
GROUPS_PER_TILE = NODE_TILE // 128      # 14
N_TILES = 7
ROWS_PAD = NODE_TILE * N_TILES          # 12544
N_CHUNKS = 4
CHUNK = N // N_CHUNKS                   # 25000
CHUNK_PAD = CHUNK + 1                   # zero row at local index 25000
PAD_IDX = CHUNK                         # gather index of the zero row
P_SLOTS = 4                             # slots per node per pass
TILE_SLOTS = NODE_TILE * P_SLOTS        # 7168 gather indices per (tile, chunk)
DUMP_NODE = ROWS_PAD - 1                # scatter target for padded entries

_PROG_CACHE = {}


def _wrap16(a):
    """[L] int -> [128, L/16] int16 in the dma_gather/scatter index layout:
    position i lives at [i % 16, i // 16], replicated to all 8 q7 cores."""
    a = np.ascontiguousarray(a.astype(np.int16))
    L = a.shape[-1]
    assert L % 16 == 0
    t = a.reshape(a.shape[:-1] + (L // 16, 16))
    t = np.swapaxes(t, -1, -2)  # [..., 16, L/16]
    reps = (1,) * (a.ndim - 1) + (8, 1)
    return np.ascontiguousarray(np.tile(t, reps))


def _slab_row(n):
    """Node index within a core -> row in the core's output slab.

    Chosen so that the per-tile result DMA is contiguous in HBM:
    node n = t*1792 + g*128 + r  ->  row t*1792 + r*14 + g."""
    t = n // NODE_TILE
    w = n % NODE_TILE
    g = w // 128
    r = w % 128
    return t * NODE_TILE + r * GROUPS_PER_TILE + g


def _host_prep(x, edge_index):
    src = np.asarray(edge_index[0], dtype=np.int64)
    dst = np.asarray(edge_index[1], dtype=np.int64)
    E = src.shape[0]

    core = dst // ROWS_PER_CORE
    n_loc = dst % ROWS_PER_CORE
    chunk = src // CHUNK
    s_loc = (src % CHUNK).astype(np.int32)

    combo = core * N_CHUNKS + chunk                       # 0..31
    gkey = combo * ROWS_PER_CORE + n_loc                  # group id
    order = np.argsort(gkey, kind="stable")
    gs = gkey[order]
    sl = s_loc[order]

    # rank of each edge within its (core, chunk, node) group
    first = np.empty(E, dtype=bool)
    first[0] = True
    np.not_equal(gs[1:], gs[:-1], out=first[1:])
    gstart = np.flatnonzero(first)
    gid = np.cumsum(first) - 1
    rank = np.arange(E, dtype=np.int64) - gstart[gid]

    deg = np.bincount(gkey, minlength=32 * ROWS_PER_CORE).reshape(32, ROWS_PER_CORE)

    e_combo = gs // ROWS_PER_CORE
    e_node = gs % ROWS_PER_CORE

    # ---- pass 1: 4 slots for every node ----
    A1 = np.full((32, ROWS_PAD, P_SLOTS), PAD_IDX, np.int16)
    m = rank < P_SLOTS
    A1[e_combo[m], e_node[m], rank[m]] = sl[m]
    # gather order: (tile, group, k, r)
    A1 = A1.reshape(32, N_TILES, GROUPS_PER_TILE, 128, P_SLOTS)
    A1 = A1.transpose(0, 1, 2, 4, 3).reshape(32, N_TILES * TILE_SLOTS)
    idx1 = _wrap16(A1).reshape(8, N_CHUNKS, 128, N_TILES * TILE_SLOTS // 16)

    # ---- overflow passes ----
    max_deg = int(deg.max())
    n_passes = max(1, -(-max_deg // P_SLOTS))  # ceil
    passes = []  # (G, idx [8,4,128,*], sidx [8,4,128,*]) per extra pass
    slab = _slab_row(np.arange(ROWS_PAD))
    for p in range(1, n_passes):
        ov = deg > P_SLOTS * p                 # [32, 12500]
        cnt = ov.sum(axis=1)
        G = int(-(-cnt.max() // 128))
        if G == 0:
            break
        npad = G * 128
        pos = np.cumsum(ov, axis=1) - 1        # position within overflow list
        sidx = np.full((32, npad), slab[DUMP_NODE], np.int16)
        ci, ni = np.nonzero(ov)
        sidx[ci, pos[ci, ni]] = slab[ni]
        Ap = np.full((32, npad, P_SLOTS), PAD_IDX, np.int16)
        mp = (rank >= P_SLOTS * p) & (rank < P_SLOTS * (p + 1))
        Ap[e_combo[mp], pos[e_combo[mp], e_node[mp]], rank[mp] - P_SLOTS * p] = sl[mp]
        Ap = Ap.reshape(32, G, 128, P_SLOTS).transpose(0, 1, 3, 2).reshape(32, npad * P_SLOTS)
        passes.append(
            (
                G,
                _wrap16(Ap).reshape(8, N_CHUNKS, 128, npad * P_SLOTS // 16),
                _wrap16(sidx).reshape(8, N_CHUNKS, 128, npad // 16),
            )
        )

    # ---- x_dev: chunks with interleaved zero rows ----
    x = np.asarray(x, dtype=np.float32)
    x_dev = np.zeros((N_CHUNKS * CHUNK_PAD, D), np.float32)
    for c in range(N_CHUNKS):
        x_dev[c * CHUNK_PAD : c * CHUNK_PAD + CHUNK] = x[c * CHUNK : (c + 1) * CHUNK]

    return x_dev, idx1, passes


def _build_program(pass_sizes):
    """pass_sizes: tuple of G (128-node groups) per overflow pass."""
    import concourse.tile as tile
    from concourse import bacc, mybir

    f32 = mybir.dt.float32
    i16 = mybir.dt.int16

    nc = bacc.Bacc(
        "TRN2",
        target_bir_lowering=False,
        debug=False,
        enable_asserts=False,
        num_devices=N_CORES,
    )
    x_t = nc.dram_tensor("x_dev", [N_CHUNKS * CHUNK_PAD, D], f32, kind="ExternalInput")
    idx1_t = [
        nc.dram_tensor(f"idx1_c{c}", [128, N_TILES * TILE_SLOTS // 16], i16, kind="ExternalInput")
        for c in range(N_CHUNKS)
    ]
    pass_t = []
    for p, G in enumerate(pass_sizes):
        gt = [
            nc.dram_tensor(f"idx_p{p}_c{c}", [128, G * 128 * P_SLOTS // 16], i16, kind="ExternalInput")
            for c in range(N_CHUNKS)
        ]
        st = [
            nc.dram_tensor(f"sidx_p{p}_c{c}", [128, G * 128 // 16], i16, kind="ExternalInput")
            for c in range(N_CHUNKS)
        ]
        pass_t.append((G, gt, st))
    out_t = nc.dram_tensor("out", [ROWS_PAD, D], f32, kind="ExternalOutput")

    x_chunks = [x_t.ap()[c * CHUNK_PAD : (c + 1) * CHUNK_PAD] for c in range(N_CHUNKS)]
    out_ap = out_t.ap()

    IDX_COLS = TILE_SLOTS // 16  # 448
    STAGE_FREE = GROUPS_PER_TILE * P_SLOTS * D  # 56*64 = 3584

    with tile.TileContext(nc) as tc:
        with (
            tc.tile_pool(name="idx", bufs=2) as idx_pool,
            tc.tile_pool(name="stage", bufs=2) as stage_pool,
            tc.tile_pool(name="tmp", bufs=2) as tmp_pool,
            tc.tile_pool(name="part", bufs=2) as part_pool,
            tc.tile_pool(name="outp", bufs=2) as out_pool,
        ):
            for t in range(N_TILES):
                parts = []
                for c in range(N_CHUNKS):
                    idx_sb = idx_pool.tile([128, IDX_COLS], i16, tag="idx")
                    nc.sync.dma_start(
                        idx_sb[:], idx1_t[c].ap()[:, t * IDX_COLS : (t + 1) * IDX_COLS]
                    )
                    st = stage_pool.tile([128, STAGE_FREE], f32, tag=f"stage{c}")
                    nc.gpsimd.dma_gather(
                        st[:].rearrange("p (s f) -> p s f", f=D),
                        x_chunks[c],
                        idx_sb[:],
                        TILE_SLOTS,
                        TILE_SLOTS,
                        D,
                        single_packet=False,
                    )
                    # reduce the 4 slot planes: view (g, k, f)
                    sv = st[:].rearrange("p (g k f) -> p g k f", k=P_SLOTS, f=D)
                    t1 = tmp_pool.tile([128, GROUPS_PER_TILE * D], f32, tag="t1")
                    t2 = tmp_pool.tile([128, GROUPS_PER_TILE * D], f32, tag="t2")
                    v1 = t1[:].rearrange("p (g f) -> p g f", f=D)
                    v2 = t2[:].rearrange("p (g f) -> p g f", f=D)
                    nc.vector.tensor_add(v1, sv[:, :, 0, :], sv[:, :, 1, :])
                    nc.vector.tensor_add(v2, sv[:, :, 2, :], sv[:, :, 3, :])
                    pc = part_pool.tile([128, GROUPS_PER_TILE * D], f32, tag=f"part{c}")
                    nc.vector.tensor_add(pc[:], t1[:], t2[:])
                    parts.append(pc)
                q1 = tmp_pool.tile([128, GROUPS_PER_TILE * D], f32, tag="t1")
                q2 = tmp_pool.tile([128, GROUPS_PER_TILE * D], f32, tag="t2")
                nc.vector.tensor_add(q1[:], parts[0][:], parts[1][:])
                nc.vector.tensor_add(q2[:], parts[2][:], parts[3][:])
                ot = out_pool.tile([128, GROUPS_PER_TILE * D], f32, tag="out")
                nc.vector.tensor_add(ot[:], q1[:], q2[:])
                # contiguous write: slab rows [t*1792, (t+1)*1792) in (r, g) order
                dview = out_ap[t * NODE_TILE : (t + 1) * NODE_TILE].rearrange(
                    "(r g) f -> r (g f)", r=128
                )
                nc.sync.dma_start(dview, ot[:])

            # overflow passes: gather, reduce 4 planes, scatter-add into slab.
            # Gathers are sub-tiled to <=14 groups so they reuse the main
            # staging slots.
            for p, (G, gt, st_t) in enumerate(pass_t):
                npad = G * 128
                for c in range(N_CHUNKS):
                    gidx = idx_pool.tile([128, npad * P_SLOTS // 16], i16, tag="ovg")
                    nc.sync.dma_start(gidx[:], gt[c].ap()[:])
                    sidx = idx_pool.tile([128, npad // 16], i16, tag="ovs")
                    nc.sync.dma_start(sidx[:], st_t[c].ap()[:])
                    pr = part_pool.tile([128, G * D], f32, tag="ovpart")
                    prv = pr[:].rearrange("p (g f) -> p g f", f=D)
                    for g0 in range(0, G, GROUPS_PER_TILE):
                        g1 = min(G, g0 + GROUPS_PER_TILE)
                        gs = g1 - g0
                        stg = stage_pool.tile([128, gs * P_SLOTS * D], f32, tag=f"stage{c}")
                        nc.gpsimd.dma_gather(
                            stg[:].rearrange("p (s f) -> p s f", f=D),
                            x_chunks[c],
                            gidx[:, g0 * 32 : g1 * 32],
                            gs * 128 * P_SLOTS,
                            gs * 128 * P_SLOTS,
                            D,
                            single_packet=False,
                        )
                        sv = stg[:].rearrange("p (g k f) -> p g k f", k=P_SLOTS, f=D)
                        t1 = tmp_pool.tile([128, gs * D], f32, tag="t1")
                        t2 = tmp_pool.tile([128, gs * D], f32, tag="t2")
                        nc.vector.tensor_add(
                            t1[:].rearrange("p (g f) -> p g f", f=D), sv[:, :, 0, :], sv[:, :, 1, :]
                        )
                        nc.vector.tensor_add(
                            t2[:].rearrange("p (g f) -> p g f", f=D), sv[:, :, 2, :], sv[:, :, 3, :]
                        )
                        nc.vector.tensor_add(prv[:, g0:g1, :], t1[:], t2[:])
                    nc.gpsimd.dma_scatter_add(
                        out_ap[:],
                        prv,
                        sidx[:],
                        npad,
                        npad,
                        D,
                        single_packet=False,
                    )

    nc.compile()
    return nc


def kernel(x, edge_index):
    from concourse import bass_utils

    x = np.asarray(x, dtype=np.float32)
    edge_index = np.asarray(edge_index)
    assert x.shape == (N, D) and edge_index.shape[1:] == (1600000,) or True

    x_dev, idx1, passes = _host_prep(x, edge_index)
    sig = tuple(G for G, _, _ in passes)
    nc = _PROG_CACHE.get(sig)
    if nc is None:
        nc = _build_program(sig)
        _PROG_CACHE[sig] = nc

    in_maps = []
    for core in range(N_CORES):
        m = {"x_dev": x_dev}
        for c in range(N_CHUNKS):
            m[f"idx1_c{c}"] = idx1[core, c]
        for p, (G, gi, si) in enumerate(passes):
            for c in range(N_CHUNKS):
                m[f"idx_p{p}_c{c}"] = gi[core, c]
                m[f"sidx_p{p}_c{c}"] = si[core, c]
        in_maps.append(m)

    res = bass_utils.run_bass_kernel_spmd(nc, in_maps, core_ids=list(range(N_CORES)))

    perm = _slab_row(np.arange(ROWS_PER_CORE))
    out = np.empty((N, D), np.float32)
    for core in range(N_CORES):
        slab = res.results[core]["out"]
        out[core * ROWS_PER_CORE : (core + 1) * ROWS_PER_CORE] = slab[perm]
    return out


# revision 5
# speedup vs baseline: 1.2408x; 1.2408x over previous
"""Trainium2 Bass kernel for GNN message passing (gather + segment_sum).

out[i] = sum_{e: dst[e]==i} x[src[e]]   with x [100000, 64] f32,
edge_index [2, 1600000] int64.

Strategy (8 NeuronCores, SPMD):
  - Destination nodes sharded across cores: core c owns dst rows
    [c*12500, (c+1)*12500), padded to a 12544-row output slab.
  - x is staged in HBM per core as x_dev [100004, 64]: 4 source chunks of
    25000 rows, each followed by one zero row (pad target). Chunking keeps
    gather indices within int16 range required by dma_gather.
  - Host pre-sorts edges by (dst-core, src-chunk, dst) and builds padded
    "slot" lists: pass 1 gives every node 4 slots per chunk; nodes with
    more than 4 in-edges from a chunk overflow into compacted extra passes
    (4 more slots each), whose partial sums are dma_scatter_add-ed (CCE
    read-modify-write) into the output slab.
  - Device: dma_gather streams 256B rows from HBM into SBUF staging tiles;
    the vector engine reduces the 4-slot planes with strided-AP adds; the
    result is written back with one contiguous DMA per node tile.
"""

import sys

if "/opt/trn_rl_repo" not in sys.path:
    sys.path.insert(0, "/opt/trn_rl_repo")

import numpy as np

N = 100000
D = 64
N_CORES = 8
ROWS_PER_CORE = N // N_CORES            # 12500
NODE_TILE = 1792                        # 14 groups of 128 nodes
GROUPS_PER_TILE = NODE_TILE // 128      # 14
N_TILES = 7
ROWS_PAD = NODE_TILE * N_TILES          # 12544
N_CHUNKS = 4
CHUNK = N // N_CHUNKS                   # 25000
CHUNK_PAD = CHUNK + 1                   # zero row at local index 25000
PAD_IDX = CHUNK                         # gather index of the zero row
P_SLOTS = 4                             # slots per node per pass
TILE_SLOTS = NODE_TILE * P_SLOTS        # 7168 gather indices per (tile, chunk)
DUMP_NODE = ROWS_PAD - 1                # scatter target for padded entries

_PROG_CACHE = {}


def _wrap16(a):
    """[L] int -> [128, L/16] int16 in the dma_gather/scatter index layout:
    position i lives at [i % 16, i // 16], replicated to all 8 q7 cores."""
    a = np.ascontiguousarray(a.astype(np.int16))
    L = a.shape[-1]
    assert L % 16 == 0
    t = a.reshape(a.shape[:-1] + (L // 16, 16))
    t = np.swapaxes(t, -1, -2)  # [..., 16, L/16]
    reps = (1,) * (a.ndim - 1) + (8, 1)
    return np.ascontiguousarray(np.tile(t, reps))


def _slab_row(n):
    """Node index within a core -> row in the core's output slab.

    Chosen so that the per-tile result DMA is contiguous in HBM:
    node n = t*1792 + g*128 + r  ->  row t*1792 + r*14 + g."""
    t = n // NODE_TILE
    w = n % NODE_TILE
    g = w // 128
    r = w % 128
    return t * NODE_TILE + r * GROUPS_PER_TILE + g


def _host_prep(x, edge_index):
    src = np.asarray(edge_index[0], dtype=np.int64)
    dst = np.asarray(edge_index[1], dtype=np.int64)
    E = src.shape[0]

    core = dst // ROWS_PER_CORE
    n_loc = dst % ROWS_PER_CORE
    chunk = src // CHUNK
    s_loc = (src % CHUNK).astype(np.int32)

    combo = core * N_CHUNKS + chunk                       # 0..31
    gkey = combo * ROWS_PER_CORE + n_loc                  # group id
    order = np.argsort(gkey, kind="stable")
    gs = gkey[order]
    sl = s_loc[order]

    # rank of each edge within its (core, chunk, node) group
    first = np.empty(E, dtype=bool)
    first[0] = True
    np.not_equal(gs[1:], gs[:-1], out=first[1:])
    gstart = np.flatnonzero(first)
    gid = np.cumsum(first) - 1
    rank = np.arange(E, dtype=np.int64) - gstart[gid]

    deg = np.bincount(gkey, minlength=32 * ROWS_PER_CORE).reshape(32, ROWS_PER_CORE)

    e_combo = gs // ROWS_PER_CORE
    e_node = gs % ROWS_PER_CORE

    # ---- pass 1: 4 slots for every node ----
    A1 = np.full((32, ROWS_PAD, P_SLOTS), PAD_IDX, np.int16)
    m = rank < P_SLOTS
    A1[e_combo[m], e_node[m], rank[m]] = sl[m]
    # gather order: (tile, group, k, r)
    A1 = A1.reshape(32, N_TILES, GROUPS_PER_TILE, 128, P_SLOTS)
    A1 = A1.transpose(0, 1, 2, 4, 3).reshape(32, N_TILES * TILE_SLOTS)
    idx1 = _wrap16(A1).reshape(8, N_CHUNKS, 128, N_TILES * TILE_SLOTS // 16)

    # ---- overflow passes ----
    max_deg = int(deg.max())
    n_passes = max(1, -(-max_deg // P_SLOTS))  # ceil
    passes = []  # (G, idx [8,4,128,*], sidx [8,4,128,*]) per extra pass
    slab = _slab_row(np.arange(ROWS_PAD))
    for p in range(1, n_passes):
        ov = deg > P_SLOTS * p                 # [32, 12500]
        cnt = ov.sum(axis=1)
        G = int(-(-cnt.max() // 128))
        if G == 0:
            break
        npad = G * 128
        pos = np.cumsum(ov, axis=1) - 1        # position within overflow list
        sidx = np.full((32, npad), slab[DUMP_NODE], np.int16)
        ci, ni = np.nonzero(ov)
        sidx[ci, pos[ci, ni]] = slab[ni]
        Ap = np.full((32, npad, P_SLOTS), PAD_IDX, np.int16)
        mp = (rank >= P_SLOTS * p) & (rank < P_SLOTS * (p + 1))
        Ap[e_combo[mp], pos[e_combo[mp], e_node[mp]], rank[mp] - P_SLOTS * p] = sl[mp]
        Ap = Ap.reshape(32, G, 128, P_SLOTS).transpose(0, 1, 3, 2).reshape(32, npad * P_SLOTS)
        passes.append(
            (
                G,
                _wrap16(Ap).reshape(8, N_CHUNKS, 128, npad * P_SLOTS // 16),
                _wrap16(sidx).reshape(8, N_CHUNKS, 128, npad // 16),
            )
        )

    # ---- x_dev: chunks with interleaved zero rows ----
    x = np.asarray(x, dtype=np.float32)
    x_dev = np.zeros((N_CHUNKS * CHUNK_PAD, D), np.float32)
    for c in range(N_CHUNKS):
        x_dev[c * CHUNK_PAD : c * CHUNK_PAD + CHUNK] = x[c * CHUNK : (c + 1) * CHUNK]

    return x_dev, idx1, passes


def _build_program(pass_sizes):
    """pass_sizes: tuple of G (128-node groups) per overflow pass."""
    import concourse.tile as tile
    from concourse import bacc, mybir

    f32 = mybir.dt.float32
    i16 = mybir.dt.int16

    nc = bacc.Bacc(
        "TRN2",
        target_bir_lowering=False,
        debug=False,
        enable_asserts=False,
        num_devices=N_CORES,
        num_swdge_queues=4,
    )
    x_t = nc.dram_tensor("x_dev", [N_CHUNKS * CHUNK_PAD, D], f32, kind="ExternalInput")
    idx1_t = [
        nc.dram_tensor(f"idx1_c{c}", [128, N_TILES * TILE_SLOTS // 16], i16, kind="ExternalInput")
        for c in range(N_CHUNKS)
    ]
    pass_t = []
    for p, G in enumerate(pass_sizes):
        gt = [
            nc.dram_tensor(f"idx_p{p}_c{c}", [128, G * 128 * P_SLOTS // 16], i16, kind="ExternalInput")
            for c in range(N_CHUNKS)
        ]
        st = [
            nc.dram_tensor(f"sidx_p{p}_c{c}", [128, G * 128 // 16], i16, kind="ExternalInput")
            for c in range(N_CHUNKS)
        ]
        pass_t.append((G, gt, st))
    out_t = nc.dram_tensor("out", [ROWS_PAD, D], f32, kind="ExternalOutput")

    x_chunks = [x_t.ap()[c * CHUNK_PAD : (c + 1) * CHUNK_PAD] for c in range(N_CHUNKS)]
    out_ap = out_t.ap()

    IDX_COLS = TILE_SLOTS // 16  # 448
    STAGE_FREE = GROUPS_PER_TILE * P_SLOTS * D  # 56*64 = 3584

    with tile.TileContext(nc) as tc:
        with (
            tc.tile_pool(name="idx", bufs=2) as idx_pool,
            tc.tile_pool(name="stage", bufs=2) as stage_pool,
            tc.tile_pool(name="tmp", bufs=2) as tmp_pool,
            tc.tile_pool(name="part", bufs=2) as part_pool,
            tc.tile_pool(name="outp", bufs=2) as out_pool,
        ):
            for t in range(N_TILES):
                parts = []
                for c in range(N_CHUNKS):
                    idx_sb = idx_pool.tile([128, IDX_COLS], i16, tag="idx")
                    nc.sync.dma_start(
                        idx_sb[:], idx1_t[c].ap()[:, t * IDX_COLS : (t + 1) * IDX_COLS]
                    )
                    st = stage_pool.tile([128, STAGE_FREE], f32, tag=f"stage{c}")
                    nc.gpsimd.dma_gather(
                        st[:].rearrange("p (s f) -> p s f", f=D),
                        x_chunks[c],
                        idx_sb[:],
                        TILE_SLOTS,
                        TILE_SLOTS,
                        D,
                        single_packet=False,
                        queue_num=c,
                    )
                    # reduce the 4 slot planes: view (g, k, f)
                    sv = st[:].rearrange("p (g k f) -> p g k f", k=P_SLOTS, f=D)
                    t1 = tmp_pool.tile([128, GROUPS_PER_TILE * D], f32, tag="t1")
                    t2 = tmp_pool.tile([128, GROUPS_PER_TILE * D], f32, tag="t2")
                    v1 = t1[:].rearrange("p (g f) -> p g f", f=D)
                    v2 = t2[:].rearrange("p (g f) -> p g f", f=D)
                    nc.vector.tensor_add(v1, sv[:, :, 0, :], sv[:, :, 1, :])
                    nc.vector.tensor_add(v2, sv[:, :, 2, :], sv[:, :, 3, :])
                    pc = part_pool.tile([128, GROUPS_PER_TILE * D], f32, tag=f"part{c}")
                    nc.vector.tensor_add(pc[:], t1[:], t2[:])
                    parts.append(pc)
                q1 = tmp_pool.tile([128, GROUPS_PER_TILE * D], f32, tag="t1")
                q2 = tmp_pool.tile([128, GROUPS_PER_TILE * D], f32, tag="t2")
                nc.vector.tensor_add(q1[:], parts[0][:], parts[1][:])
                nc.vector.tensor_add(q2[:], parts[2][:], parts[3][:])
                ot = out_pool.tile([128, GROUPS_PER_TILE * D], f32, tag="out")
                nc.vector.tensor_add(ot[:], q1[:], q2[:])
                # contiguous write: slab rows [t*1792, (t+1)*1792) in (r, g) order
                dview = out_ap[t * NODE_TILE : (t + 1) * NODE_TILE].rearrange(
                    "(r g) f -> r (g f)", r=128
                )
                nc.sync.dma_start(dview, ot[:])

            # overflow passes: gather, reduce 4 planes, scatter-add into slab.
            # Gathers are sub-tiled to <=14 groups so they reuse the main
            # staging slots.
            for p, (G, gt, st_t) in enumerate(pass_t):
                npad = G * 128
                for c in range(N_CHUNKS):
                    gidx = idx_pool.tile([128, npad * P_SLOTS // 16], i16, tag="ovg")
                    nc.sync.dma_start(gidx[:], gt[c].ap()[:])
                    sidx = idx_pool.tile([128, npad // 16], i16, tag="ovs")
                    nc.sync.dma_start(sidx[:], st_t[c].ap()[:])
                    pr = part_pool.tile([128, G * D], f32, tag="ovpart")
                    prv = pr[:].rearrange("p (g f) -> p g f", f=D)
                    for g0 in range(0, G, GROUPS_PER_TILE):
                        g1 = min(G, g0 + GROUPS_PER_TILE)
                        gs = g1 - g0
                        stg = stage_pool.tile([128, gs * P_SLOTS * D], f32, tag=f"stage{c}")
                        nc.gpsimd.dma_gather(
                            stg[:].rearrange("p (s f) -> p s f", f=D),
                            x_chunks[c],
                            gidx[:, g0 * 32 : g1 * 32],
                            gs * 128 * P_SLOTS,
                            gs * 128 * P_SLOTS,
                            D,
                            single_packet=False,
                            queue_num=c,
                        )
                        sv = stg[:].rearrange("p (g k f) -> p g k f", k=P_SLOTS, f=D)
                        t1 = tmp_pool.tile([128, gs * D], f32, tag="t1")
                        t2 = tmp_pool.tile([128, gs * D], f32, tag="t2")
                        nc.vector.tensor_add(
                            t1[:].rearrange("p (g f) -> p g f", f=D), sv[:, :, 0, :], sv[:, :, 1, :]
                        )
                        nc.vector.tensor_add(
                            t2[:].rearrange("p (g f) -> p g f", f=D), sv[:, :, 2, :], sv[:, :, 3, :]
                        )
                        nc.vector.tensor_add(prv[:, g0:g1, :], t1[:], t2[:])
                    nc.gpsimd.dma_scatter_add(
                        out_ap[:],
                        prv,
                        sidx[:],
                        npad,
                        npad,
                        D,
                        single_packet=False,
                        queue_num=c,
                    )

    nc.compile()
    return nc


def kernel(x, edge_index):
    from concourse import bass_utils

    x = np.asarray(x, dtype=np.float32)
    edge_index = np.asarray(edge_index)
    assert x.shape == (N, D) and edge_index.shape[1:] == (1600000,) or True

    x_dev, idx1, passes = _host_prep(x, edge_index)
    sig = tuple(G for G, _, _ in passes)
    nc = _PROG_CACHE.get(sig)
    if nc is None:
        nc = _build_program(sig)
        _PROG_CACHE[sig] = nc

    in_maps = []
    for core in range(N_CORES):
        m = {"x_dev": x_dev}
        for c in range(N_CHUNKS):
            m[f"idx1_c{c}"] = idx1[core, c]
        for p, (G, gi, si) in enumerate(passes):
            for c in range(N_CHUNKS):
                m[f"idx_p{p}_c{c}"] = gi[core, c]
                m[f"sidx_p{p}_c{c}"] = si[core, c]
        in_maps.append(m)

    res = bass_utils.run_bass_kernel_spmd(nc, in_maps, core_ids=list(range(N_CORES)))

    perm = _slab_row(np.arange(ROWS_PER_CORE))
    out = np.empty((N, D), np.float32)
    for core in range(N_CORES):
        slab = res.results[core]["out"]
        out[core * ROWS_PER_CORE : (core + 1) * ROWS_PER_CORE] = slab[perm]
    return out


# revision 7
# speedup vs baseline: 2.0665x; 1.6655x over previous
"""Trainium2 Bass kernel for GNN message passing (gather + segment_sum).

out[i] = sum_{e: dst[e]==i} x[src[e]]   with x [100000, 64] f32,
edge_index [2, 1600000] int64.

Strategy (8 NeuronCores, SPMD):
  - Destination nodes sharded across cores: core c owns dst rows
    [c*12500, (c+1)*12500), padded to a 12544-row output slab.
  - x is staged in HBM per core as x_dev [100004, 64]: 4 source chunks of
    25000 rows, each followed by one zero row (pad target). Chunking keeps
    gather indices within int16 range required by dma_gather.
  - Host pre-sorts edges by (dst-core, src-chunk, dst) and builds padded
    "slot" lists: pass 1 gives every node 4 slots per chunk; nodes with
    more than 4 in-edges from a chunk overflow into compacted extra passes
    (4 more slots each), whose partial sums are dma_scatter_add-ed (CCE
    read-modify-write) into the output slab.
  - Device: dma_gather streams 256B rows from HBM into SBUF staging tiles;
    the vector engine reduces the 4-slot planes with strided-AP adds; the
    result is written back with one contiguous DMA per node tile.
"""

import sys

if "/opt/trn_rl_repo" not in sys.path:
    sys.path.insert(0, "/opt/trn_rl_repo")

import numpy as np

N = 100000
D = 64
N_CORES = 8
ROWS_PER_CORE = N // N_CORES            # 12500
NODE_TILE = 1792                        # 14 groups of 128 nodes
GROUPS_PER_TILE = NODE_TILE // 128      # 14
N_TILES = 7
ROWS_PAD = NODE_TILE * N_TILES          # 12544
N_CHUNKS = 4
CHUNK = N // N_CHUNKS                   # 25000
CHUNK_PAD = CHUNK + 1                   # zero row at local index 25000
PAD_IDX = CHUNK                         # gather index of the zero row
P_SLOTS = 4                             # slots per node per pass
TILE_SLOTS = NODE_TILE * P_SLOTS        # 7168 gather indices per (tile, chunk)
DUMP_NODE = ROWS_PAD - 1                # scatter target for padded entries

_PROG_CACHE = {}


def _wrap16(a):
    """[L] int -> [128, L/16] int16 in the dma_gather/scatter index layout:
    position i lives at [i % 16, i // 16], replicated to all 8 q7 cores."""
    a = np.ascontiguousarray(a.astype(np.int16))
    L = a.shape[-1]
    assert L % 16 == 0
    t = a.reshape(a.shape[:-1] + (L // 16, 16))
    t = np.swapaxes(t, -1, -2)  # [..., 16, L/16]
    reps = (1,) * (a.ndim - 1) + (8, 1)
    return np.ascontiguousarray(np.tile(t, reps))


def _slab_row(n):
    """Node index within a core -> row in the core's output slab.

    Chosen so that the per-tile result DMA is contiguous in HBM:
    node n = t*1792 + g*128 + r  ->  row t*1792 + r*14 + g."""
    t = n // NODE_TILE
    w = n % NODE_TILE
    g = w // 128
    r = w % 128
    return t * NODE_TILE + r * GROUPS_PER_TILE + g


def _host_prep(x, edge_index):
    src = np.asarray(edge_index[0], dtype=np.int64)
    dst = np.asarray(edge_index[1], dtype=np.int64)
    E = src.shape[0]

    core = dst // ROWS_PER_CORE
    n_loc = dst % ROWS_PER_CORE
    chunk = src // CHUNK
    s_loc = (src % CHUNK).astype(np.int32)

    combo = core * N_CHUNKS + chunk                       # 0..31
    gkey = combo * ROWS_PER_CORE + n_loc                  # group id
    order = np.argsort(gkey, kind="stable")
    gs = gkey[order]
    sl = s_loc[order]

    # rank of each edge within its (core, chunk, node) group
    first = np.empty(E, dtype=bool)
    first[0] = True
    np.not_equal(gs[1:], gs[:-1], out=first[1:])
    gstart = np.flatnonzero(first)
    gid = np.cumsum(first) - 1
    rank = np.arange(E, dtype=np.int64) - gstart[gid]

    deg = np.bincount(gkey, minlength=32 * ROWS_PER_CORE).reshape(32, ROWS_PER_CORE)

    e_combo = gs // ROWS_PER_CORE
    e_node = gs % ROWS_PER_CORE

    # ---- pass 1: 4 slots for every node ----
    A1 = np.full((32, ROWS_PAD, P_SLOTS), PAD_IDX, np.int16)
    m = rank < P_SLOTS
    A1[e_combo[m], e_node[m], rank[m]] = sl[m]
    # gather order: (tile, group, k, r)
    A1 = A1.reshape(32, N_TILES, GROUPS_PER_TILE, 128, P_SLOTS)
    A1 = A1.transpose(0, 1, 2, 4, 3).reshape(32, N_TILES * TILE_SLOTS)
    idx1 = _wrap16(A1).reshape(8, N_CHUNKS, 128, N_TILES * TILE_SLOTS // 16)

    # ---- overflow passes ----
    max_deg = int(deg.max())
    n_passes = max(1, -(-max_deg // P_SLOTS))  # ceil
    passes = []  # (G, idx [8,4,128,*], sidx [8,4,128,*]) per extra pass
    slab = _slab_row(np.arange(ROWS_PAD))
    for p in range(1, n_passes):
        ov = deg > P_SLOTS * p                 # [32, 12500]
        cnt = ov.sum(axis=1)
        G = int(-(-cnt.max() // 128))
        if G == 0:
            break
        npad = G * 128
        pos = np.cumsum(ov, axis=1) - 1        # position within overflow list
        sidx = np.full((32, npad), slab[DUMP_NODE], np.int16)
        ci, ni = np.nonzero(ov)
        sidx[ci, pos[ci, ni]] = slab[ni]
        Ap = np.full((32, npad, P_SLOTS), PAD_IDX, np.int16)
        mp = (rank >= P_SLOTS * p) & (rank < P_SLOTS * (p + 1))
        Ap[e_combo[mp], pos[e_combo[mp], e_node[mp]], rank[mp] - P_SLOTS * p] = sl[mp]
        Ap = Ap.reshape(32, G, 128, P_SLOTS).transpose(0, 1, 3, 2).reshape(32, npad * P_SLOTS)
        passes.append(
            (
                G,
                _wrap16(Ap).reshape(8, N_CHUNKS, 128, npad * P_SLOTS // 16),
                _wrap16(sidx).reshape(8, N_CHUNKS, 128, npad // 16),
            )
        )

    # ---- x_dev: chunks with interleaved zero rows ----
    x = np.asarray(x, dtype=np.float32)
    x_dev = np.zeros((N_CHUNKS * CHUNK_PAD, D), np.float32)
    for c in range(N_CHUNKS):
        x_dev[c * CHUNK_PAD : c * CHUNK_PAD + CHUNK] = x[c * CHUNK : (c + 1) * CHUNK]

    return x_dev, idx1, passes


def _build_program(pass_sizes):
    """pass_sizes: tuple of G (128-node groups) per overflow pass."""
    import concourse.tile as tile
    from concourse import bacc, mybir

    f32 = mybir.dt.float32
    i16 = mybir.dt.int16

    nc = bacc.Bacc(
        "TRN2",
        target_bir_lowering=False,
        debug=False,
        enable_asserts=False,
        num_devices=N_CORES,
        num_swdge_queues=4,
    )
    x_t = nc.dram_tensor("x_dev", [N_CHUNKS * CHUNK_PAD, D], f32, kind="ExternalInput")
    idx1_t = [
        nc.dram_tensor(f"idx1_c{c}", [128, N_TILES * TILE_SLOTS // 16], i16, kind="ExternalInput")
        for c in range(N_CHUNKS)
    ]
    pass_t = []
    for p, G in enumerate(pass_sizes):
        gt = [
            nc.dram_tensor(f"idx_p{p}_c{c}", [128, G * 128 * P_SLOTS // 16], i16, kind="ExternalInput")
            for c in range(N_CHUNKS)
        ]
        st = [
            nc.dram_tensor(f"sidx_p{p}_c{c}", [128, G * 128 // 16], i16, kind="ExternalInput")
            for c in range(N_CHUNKS)
        ]
        pass_t.append((G, gt, st))
    out_t = nc.dram_tensor("out", [ROWS_PAD, D], f32, kind="ExternalOutput")

    x_chunks = [x_t.ap()[c * CHUNK_PAD : (c + 1) * CHUNK_PAD] for c in range(N_CHUNKS)]
    out_ap = out_t.ap()

    IDX_COLS = TILE_SLOTS // 16  # 448
    STAGE_FREE = GROUPS_PER_TILE * P_SLOTS * D  # 56*64 = 3584

    with tile.TileContext(nc) as tc:
        with (
            tc.tile_pool(name="idxr", bufs=1) as idxr_pool,
            tc.tile_pool(name="stage", bufs=2) as stage_pool,
            tc.tile_pool(name="tmp", bufs=2) as tmp_pool,
            tc.tile_pool(name="part", bufs=1) as part_pool,
            tc.tile_pool(name="outp", bufs=2) as out_pool,
        ):
            # resident index tiles (loaded once; gathers slice them)
            idx1_sb = []
            for c in range(N_CHUNKS):
                t_ = idxr_pool.tile([128, N_TILES * TILE_SLOTS // 16], i16, tag=f"idx1_{c}")
                nc.sync.dma_start(t_[:], idx1_t[c].ap()[:])
                idx1_sb.append(t_)
            ov_sb = []
            for p, (G, gt, st_t) in enumerate(pass_t):
                npad = G * 128
                row = []
                for c in range(N_CHUNKS):
                    gi = idxr_pool.tile([128, npad * P_SLOTS // 16], i16, tag=f"ovg{p}_{c}")
                    nc.sync.dma_start(gi[:], gt[c].ap()[:])
                    si = idxr_pool.tile([128, npad // 16], i16, tag=f"ovs{p}_{c}")
                    nc.sync.dma_start(si[:], st_t[c].ap()[:])
                    row.append((gi, si))
                ov_sb.append(row)

            # overflow gathers + reduces + scatter-adds first (out is
            # zero-initialized; scatters land before the accumulating tile writes)
            for p, (G, gt, st_t) in enumerate(pass_t):
                npad = G * 128
                for c in range(N_CHUNKS):
                    gidx, sidx = ov_sb[p][c]
                    pr = part_pool.tile([128, G * D], f32, tag="ovpart")
                    prv = pr[:].rearrange("p (g f) -> p g f", f=D)
                    for g0 in range(0, G, GROUPS_PER_TILE):
                        g1 = min(G, g0 + GROUPS_PER_TILE)
                        gs = g1 - g0
                        stg = stage_pool.tile([128, gs * P_SLOTS * D], f32, tag=f"stage{c}")
                        nc.gpsimd.dma_gather(
                            stg[:].rearrange("p (s f) -> p s f", f=D),
                            x_chunks[c],
                            gidx[:, g0 * 32 : g1 * 32],
                            gs * 128 * P_SLOTS,
                            gs * 128 * P_SLOTS,
                            D,
                            single_packet=False,
                            queue_num=c,
                        )
                        sv = stg[:].rearrange("p (g k f) -> p g k f", k=P_SLOTS, f=D)
                        t1 = tmp_pool.tile([128, gs * D], f32, tag="t1")
                        t2 = tmp_pool.tile([128, gs * D], f32, tag="t2")
                        nc.any.tensor_tensor(
                            t1[:].rearrange("p (g f) -> p g f", f=D), sv[:, :, 0, :], sv[:, :, 1, :],
                            op=mybir.AluOpType.add)
                        nc.any.tensor_tensor(
                            t2[:].rearrange("p (g f) -> p g f", f=D), sv[:, :, 2, :], sv[:, :, 3, :],
                            op=mybir.AluOpType.add)
                        nc.any.tensor_tensor(prv[:, g0:g1, :], t1[:], t2[:], op=mybir.AluOpType.add)
                    nc.gpsimd.dma_scatter_add(
                        out_ap[:],
                        prv,
                        sidx[:],
                        npad,
                        npad,
                        D,
                        single_packet=False,
                        queue_num=c,
                    )

            for t in range(N_TILES):
                parts = []
                for c in range(N_CHUNKS):
                    st = stage_pool.tile([128, STAGE_FREE], f32, tag=f"stage{c}")
                    nc.gpsimd.dma_gather(
                        st[:].rearrange("p (s f) -> p s f", f=D),
                        x_chunks[c],
                        idx1_sb[c][:, t * IDX_COLS : (t + 1) * IDX_COLS],
                        TILE_SLOTS,
                        TILE_SLOTS,
                        D,
                        single_packet=False,
                        queue_num=c,
                    )
                    sv = st[:].rearrange("p (g k f) -> p g k f", k=P_SLOTS, f=D)
                    t1 = tmp_pool.tile([128, GROUPS_PER_TILE * D], f32, tag="t1")
                    t2 = tmp_pool.tile([128, GROUPS_PER_TILE * D], f32, tag="t2")
                    nc.any.tensor_tensor(
                        t1[:].rearrange("p (g f) -> p g f", f=D), sv[:, :, 0, :], sv[:, :, 1, :],
                        op=mybir.AluOpType.add)
                    nc.any.tensor_tensor(
                        t2[:].rearrange("p (g f) -> p g f", f=D), sv[:, :, 2, :], sv[:, :, 3, :],
                        op=mybir.AluOpType.add)
                    pc = part_pool.tile([128, GROUPS_PER_TILE * D], f32, tag=f"part{c}")
                    nc.any.tensor_tensor(pc[:], t1[:], t2[:], op=mybir.AluOpType.add)
                    parts.append(pc)
                q1 = tmp_pool.tile([128, GROUPS_PER_TILE * D], f32, tag="t1")
                q2 = tmp_pool.tile([128, GROUPS_PER_TILE * D], f32, tag="t2")
                nc.any.tensor_tensor(q1[:], parts[0][:], parts[1][:], op=mybir.AluOpType.add)
                nc.any.tensor_tensor(q2[:], parts[2][:], parts[3][:], op=mybir.AluOpType.add)
                ot = out_pool.tile([128, GROUPS_PER_TILE * D], f32, tag="out")
                nc.any.tensor_tensor(ot[:], q1[:], q2[:], op=mybir.AluOpType.add)
                dview = out_ap[t * NODE_TILE : (t + 1) * NODE_TILE].rearrange(
                    "(r g) f -> r (g f)", r=128
                )
                nc.gpsimd.dma_start(dview, ot[:], accum_op=mybir.AluOpType.add)

    nc.compile()
    return nc


def kernel(x, edge_index):
    from concourse import bass_utils

    x = np.asarray(x, dtype=np.float32)
    edge_index = np.asarray(edge_index)
    assert x.shape == (N, D) and edge_index.shape[1:] == (1600000,) or True

    x_dev, idx1, passes = _host_prep(x, edge_index)
    sig = tuple(G for G, _, _ in passes)
    nc = _PROG_CACHE.get(sig)
    if nc is None:
        nc = _build_program(sig)
        _PROG_CACHE[sig] = nc

    in_maps = []
    for core in range(N_CORES):
        m = {"x_dev": x_dev}
        for c in range(N_CHUNKS):
            m[f"idx1_c{c}"] = idx1[core, c]
        for p, (G, gi, si) in enumerate(passes):
            for c in range(N_CHUNKS):
                m[f"idx_p{p}_c{c}"] = gi[core, c]
                m[f"sidx_p{p}_c{c}"] = si[core, c]
        in_maps.append(m)

    res = bass_utils.run_bass_kernel_spmd(nc, in_maps, core_ids=list(range(N_CORES)))

    perm = _slab_row(np.arange(ROWS_PER_CORE))
    out = np.empty((N, D), np.float32)
    for core in range(N_CORES):
        slab = res.results[core]["out"]
        out[core * ROWS_PER_CORE : (core + 1) * ROWS_PER_CORE] = slab[perm]
    return out


# revision 9
# speedup vs baseline: 2.7197x; 1.3161x over previous
"""Trainium2 Bass kernel for GNN message passing (gather + segment_sum).

out[i] = sum_{e: dst[e]==i} x[src[e]]   with x [100000, 64] f32,
edge_index [2, 1600000] int64.

Strategy (8 NeuronCores, SPMD):
  - Destination nodes sharded across cores: core c owns dst rows
    [c*12500, (c+1)*12500), padded to a 12544-row output slab whose row
    order is chosen so every device write is contiguous (host un-permutes).
  - Source nodes are split into 4 chunks of 25000 rows so dma_gather's
    int16 indices stay in range. Each chunk region in HBM also carries a
    zero pad row and per-level scratch rows (see below).
  - Host sorts edges by (dst-core, src-chunk, dst) and assigns each node
    4 "slots" per chunk per level: level 1 holds in-edge ranks 0-3 (or
    0-2 plus a pointer), level L>=2 holds ranks 3(L-1)..3L-1 plus a
    pointer to level L+1. A pointer is the scratch row where the deeper
    level's partial sum is written, so high-degree nodes chain through
    levels and no scatter operation is ever needed.
  - Device: levels run deepest-first; each is a dma_gather (256B rows,
    descriptor generation spread over the 4 SWDGE queues = 4 Q7 core
    pairs), a strided 4-plane vector-engine reduction, and one contiguous
    DMA (scratch rows for levels >= 2, output slab rows for level 1).
"""

import sys

if "/opt/trn_rl_repo" not in sys.path:
    sys.path.insert(0, "/opt/trn_rl_repo")

import numpy as np

N = 100000
D = 64
N_CORES = 8
ROWS_PER_CORE = N // N_CORES            # 12500
NODE_TILE = 1792                        # 14 groups of 128 nodes
GROUPS_PER_TILE = NODE_TILE // 128      # 14
N_TILES = 7
ROWS_PAD = NODE_TILE * N_TILES          # 12544
N_CHUNKS = 4
CHUNK = N // N_CHUNKS                   # 25000
PAD_IDX = CHUNK                         # gather index of the zero row
P_SLOTS = 4
TILE_SLOTS = NODE_TILE * P_SLOTS        # 7168 gather indices per (tile, chunk)

_PROG_CACHE = {}


def _wrap16(a):
    """[..., L] int -> [..., 128, L/16] int16 in the dma_gather index layout:
    position i at [i % 16, i // 16], replicated to all 4 queue core pairs."""
    a = np.ascontiguousarray(a.astype(np.int16))
    L = a.shape[-1]
    assert L % 16 == 0
    t = a.reshape(a.shape[:-1] + (L // 16, 16))
    t = np.swapaxes(t, -1, -2)
    reps = (1,) * (a.ndim - 1) + (8, 1)
    return np.ascontiguousarray(np.tile(t, reps))


def _slab_row(n):
    """Node index within a core -> output slab row (makes tile DMAs contiguous)."""
    t = n // NODE_TILE
    w = n % NODE_TILE
    g = w // 128
    r = w % 128
    return t * NODE_TILE + r * GROUPS_PER_TILE + g


def _gather_order(A):
    """[..., nodes(G*128), 4] slots -> flat gather list order (g, k, r)."""
    G = A.shape[-2] // 128
    A = A.reshape(A.shape[:-2] + (G, 128, P_SLOTS))
    A = np.swapaxes(A, -1, -2)  # (..., G, 4, 128)
    return A.reshape(A.shape[:-3] + (G * 128 * P_SLOTS,))


def _host_prep(x, edge_index):
    src = np.asarray(edge_index[0], dtype=np.int64)
    dst = np.asarray(edge_index[1], dtype=np.int64)
    E = src.shape[0]

    core = dst // ROWS_PER_CORE
    n_loc = dst % ROWS_PER_CORE
    chunk = src // CHUNK
    s_loc = (src % CHUNK).astype(np.int32)

    combo = core * N_CHUNKS + chunk
    gkey = combo * ROWS_PER_CORE + n_loc
    order = np.argsort(gkey, kind="stable")
    gs = gkey[order]
    sl = s_loc[order]

    first = np.empty(E, dtype=bool)
    first[0] = True
    np.not_equal(gs[1:], gs[:-1], out=first[1:])
    gstart = np.flatnonzero(first)
    gid = np.cumsum(first) - 1
    rank = np.arange(E, dtype=np.int64) - gstart[gid]

    deg = np.bincount(gkey, minlength=32 * ROWS_PER_CORE).reshape(32, ROWS_PER_CORE)
    e_combo = gs // ROWS_PER_CORE
    e_node = gs % ROWS_PER_CORE
    e_deg = deg[e_combo, e_node]

    # level of each edge: min(rank//3 + 1, n_levels(deg));
    # n_levels(d) = 1 if d<=4 else 1 + ceil((d-4)/3)
    e_nlvl = np.where(e_deg <= 4, 1, 1 + (np.maximum(e_deg, 5) - 4 + 2) // 3)
    e_lvl = np.minimum(rank // 3 + 1, e_nlvl)
    e_slot = rank - 3 * (e_lvl - 1)

    max_lvl = int(e_lvl.max()) if E else 1

    # level membership/positions, sizes (common across combos), scratch offsets
    lv_pos = [None, None]
    lv_S = [None, None]
    for lv in range(2, max_lvl + 1):
        m = deg > 3 * lv - 2          # [32, 12500]
        cnt = m.sum(axis=1)
        G = int(-(-cnt.max() // 128))
        lv_pos.append(np.cumsum(m, axis=1) - 1)
        lv_S.append(G * 128)

    off = [None, None]
    cur = CHUNK + 1
    for lv in range(2, max_lvl + 1):
        off.append(cur)
        cur += lv_S[lv]
    chunk_region = cur
    assert chunk_region <= 32767, chunk_region

    # ---- slot tables ----
    A = [None, np.full((32, ROWS_PAD, P_SLOTS), PAD_IDX, np.int16)]
    for lv in range(2, max_lvl + 1):
        A.append(np.full((32, lv_S[lv], P_SLOTS), PAD_IDX, np.int16))

    for lv in range(1, max_lvl + 1):
        m = e_lvl == lv
        ec, en, ek, ev = e_combo[m], e_node[m], e_slot[m], sl[m]
        if lv == 1:
            A[1][ec, en, ek] = ev
        else:
            A[lv][ec, lv_pos[lv][ec, en], ek] = ev

    # pointer slots: node at level lv that continues to lv+1 -> slot 3 = scratch
    # row; scratch rows are stored r-major: pos p -> (p % 128) * G + p // 128
    for lv in range(1, max_lvl):
        deeper = deg > 3 * lv + 1
        ci, ni = np.nonzero(deeper)
        p_ = lv_pos[lv + 1][ci, ni]
        G_ = lv_S[lv + 1] // 128
        ptr = off[lv + 1] + (p_ % 128) * G_ + p_ // 128
        if lv == 1:
            A[1][ci, ni, 3] = ptr
        else:
            A[lv][ci, lv_pos[lv][ci, ni], 3] = ptr

    idx1 = _wrap16(_gather_order(A[1])).reshape(8, N_CHUNKS, 128, -1)
    lv_idx = [None, None]
    for lv in range(2, max_lvl + 1):
        lv_idx.append(_wrap16(_gather_order(A[lv])).reshape(8, N_CHUNKS, 128, -1))

    # ---- x_dev with per-chunk scratch regions ----
    x = np.asarray(x, dtype=np.float32)
    x_dev = np.zeros((N_CHUNKS * chunk_region, D), np.float32)
    for c in range(N_CHUNKS):
        x_dev[c * chunk_region : c * chunk_region + CHUNK] = x[c * CHUNK : (c + 1) * CHUNK]

    sizes = tuple(lv_S[2:])
    return x_dev, idx1, lv_idx, sizes, chunk_region


def _build_program(sizes, chunk_region):
    """sizes: scratch rows per level (level 2 first)."""
    import concourse.tile as tile
    from concourse import bacc, mybir

    f32 = mybir.dt.float32
    i16 = mybir.dt.int16
    add = mybir.AluOpType.add

    nc = bacc.Bacc(
        "TRN2",
        target_bir_lowering=False,
        debug=False,
        enable_asserts=False,
        num_devices=N_CORES,
        num_swdge_queues=4,
    )
    x_t = nc.dram_tensor("x_dev", [N_CHUNKS * chunk_region, D], f32, kind="ExternalInput")
    idx1_t = [
        nc.dram_tensor(f"idx1_c{c}", [128, N_TILES * TILE_SLOTS // 16], i16, kind="ExternalInput")
        for c in range(N_CHUNKS)
    ]
    lv_t = []
    for li, S in enumerate(sizes):
        lv_t.append(
            [
                nc.dram_tensor(f"idx_l{li}_c{c}", [128, S * P_SLOTS // 16], i16, kind="ExternalInput")
                for c in range(N_CHUNKS)
            ]
        )
    out_t = nc.dram_tensor("out", [ROWS_PAD, D], f32, kind="ExternalOutput")

    regions = [x_t.ap()[c * chunk_region : (c + 1) * chunk_region] for c in range(N_CHUNKS)]
    out_ap = out_t.ap()

    offs = []
    cur = CHUNK + 1
    for S in sizes:
        offs.append(cur)
        cur += S

    IDX_COLS = TILE_SLOTS // 16
    STAGE_FREE = GROUPS_PER_TILE * P_SLOTS * D

    with tile.TileContext(nc) as tc:
        with (
            tc.tile_pool(name="idxr", bufs=1) as idxr_pool,
            tc.tile_pool(name="stage", bufs=2) as stage_pool,
            tc.tile_pool(name="tmp", bufs=2) as tmp_pool,
            tc.tile_pool(name="part", bufs=1) as part_pool,
            tc.tile_pool(name="outp", bufs=2) as out_pool,
        ):
            def reduce4(stg, gsz, dst_view):
                sv = stg[:].rearrange("p (g k f) -> p g k f", k=P_SLOTS, f=D)
                t1 = tmp_pool.tile([128, GROUPS_PER_TILE * D], f32, tag="t1")
                t2 = tmp_pool.tile([128, GROUPS_PER_TILE * D], f32, tag="t2")
                v1 = t1[:, : gsz * D].rearrange("p (g f) -> p g f", f=D)
                v2 = t2[:, : gsz * D].rearrange("p (g f) -> p g f", f=D)
                nc.any.tensor_tensor(v1, sv[:, :, 0, :], sv[:, :, 1, :], op=add)
                nc.any.tensor_tensor(v2, sv[:, :, 2, :], sv[:, :, 3, :], op=add)
                nc.any.tensor_tensor(dst_view, v1, v2, op=add)

            idx1_sb = []
            for c in range(N_CHUNKS):
                t_ = idxr_pool.tile([128, N_TILES * TILE_SLOTS // 16], i16, tag=f"idx1_{c}")
                nc.sync.dma_start(t_[:], idx1_t[c].ap()[:])
                idx1_sb.append(t_)
            lv_sb = []
            for li, S in enumerate(sizes):
                row = []
                for c in range(N_CHUNKS):
                    t_ = idxr_pool.tile([128, S * P_SLOTS // 16], i16, tag=f"lv{li}_{c}")
                    nc.sync.dma_start(t_[:], lv_t[li][c].ap()[:])
                    row.append(t_)
                lv_sb.append(row)

            # levels, deepest first: gather -> reduce -> contiguous scratch write
            for li in range(len(sizes) - 1, -1, -1):
                S = sizes[li]
                G = S // 128
                for c in range(N_CHUNKS):
                    pr = part_pool.tile([128, G * D], f32, tag="lvpart")
                    prv = pr[:].rearrange("p (g f) -> p g f", f=D)
                    for g0 in range(0, G, GROUPS_PER_TILE):
                        g1 = min(G, g0 + GROUPS_PER_TILE)
                        gsz = g1 - g0
                        stg = stage_pool.tile([128, gsz * P_SLOTS * D], f32, tag=f"stage{c}")
                        nc.gpsimd.dma_gather(
                            stg[:].rearrange("p (s f) -> p s f", f=D),
                            regions[c],
                            lv_sb[li][c][:, g0 * 32 : g1 * 32],
                            gsz * 128 * P_SLOTS,
                            gsz * 128 * P_SLOTS,
                            D,
                            single_packet=False,
                            queue_num=c,
                        )
                        reduce4(stg, gsz, prv[:, g0:g1, :])
                    dview = regions[c][offs[li] : offs[li] + S].rearrange(
                        "(r g) f -> r (g f)", r=128
                    )
                    nc.sync.dma_start(dview, pr[:])

            # level 1: main tiles
            for t in range(N_TILES):
                parts = []
                for c in range(N_CHUNKS):
                    st = stage_pool.tile([128, STAGE_FREE], f32, tag=f"stage{c}")
                    nc.gpsimd.dma_gather(
                        st[:].rearrange("p (s f) -> p s f", f=D),
                        regions[c],
                        idx1_sb[c][:, t * IDX_COLS : (t + 1) * IDX_COLS],
                        TILE_SLOTS,
                        TILE_SLOTS,
                        D,
                        single_packet=False,
                        queue_num=c,
                    )
                    pc = part_pool.tile([128, GROUPS_PER_TILE * D], f32, tag=f"part{c}")
                    reduce4(st, GROUPS_PER_TILE, pc[:].rearrange("p (g f) -> p g f", f=D))
                    parts.append(pc)
                q1 = tmp_pool.tile([128, GROUPS_PER_TILE * D], f32, tag="t1")
                q2 = tmp_pool.tile([128, GROUPS_PER_TILE * D], f32, tag="t2")
                nc.any.tensor_tensor(q1[:], parts[0][:], parts[1][:], op=add)
                nc.any.tensor_tensor(q2[:], parts[2][:], parts[3][:], op=add)
                ot = out_pool.tile([128, GROUPS_PER_TILE * D], f32, tag="out")
                nc.any.tensor_tensor(ot[:], q1[:], q2[:], op=add)
                dview = out_ap[t * NODE_TILE : (t + 1) * NODE_TILE].rearrange(
                    "(r g) f -> r (g f)", r=128
                )
                nc.sync.dma_start(dview, ot[:])

    nc.compile()
    return nc


def kernel(x, edge_index):
    from concourse import bass_utils

    x = np.asarray(x, dtype=np.float32)
    edge_index = np.asarray(edge_index)

    x_dev, idx1, lv_idx, sizes, chunk_region = _host_prep(x, edge_index)
    sig = (sizes, chunk_region)
    nc = _PROG_CACHE.get(sig)
    if nc is None:
        nc = _build_program(sizes, chunk_region)
        _PROG_CACHE[sig] = nc

    in_maps = []
    for core in range(N_CORES):
        m = {"x_dev": x_dev}
        for c in range(N_CHUNKS):
            m[f"idx1_c{c}"] = idx1[core, c]
        for li in range(len(sizes)):
            for c in range(N_CHUNKS):
                m[f"idx_l{li}_c{c}"] = lv_idx[li + 2][core, c]
        in_maps.append(m)

    res = bass_utils.run_bass_kernel_spmd(nc, in_maps, core_ids=list(range(N_CORES)))

    perm = _slab_row(np.arange(ROWS_PER_CORE))
    out = np.empty((N, D), np.float32)
    for core in range(N_CORES):
        slab = res.results[core]["out"]
        out[core * ROWS_PER_CORE : (core + 1) * ROWS_PER_CORE] = slab[perm]
    return out
